# revision 50
# baseline (speedup 1.0000x reference)
"""Distributed Trainium2 kernel for nn_Attention_21990232555717.

Reference (per batch element a, seq s=1024, model dim c=1024, 16 heads):
    qkv = x @ w_qkv                       # (s, 3072)
    scores = q @ k.T * (1/sqrt(1024))     # (h, s, s)
    attn = softmax(scores, axis=HEADS)    # normalize across the 16 heads
    out = attn @ v -> (s, 1024) @ w_out + b_out

Sharding: pure data parallel - batch (8) across 8 cores, weights replicated.

Per-core dataflow, all bf16 on the matmul paths (inputs converted to bf16
and x pre-transposed on the host, f32 accumulation in PSUM):
  xT   (c, s)  loaded directly (host supplies x^T)
  QKT  (f, s)  = w^T @ x^T    Q tiles 0..7, K tiles 8..15 (2 heads/tile)
  Vb   (s, f)  = x @ w_v
  per q-block of 128 (8 blocks), per head pair: scoresT (k,q) for both
    heads land in one 4-bank PSUM tile -> ONE exp over FD=2048 into
    E[128, 16, 1024] (heads interleaved)
    D = sum_h E via one DVE tree; attn = E * recip(D) (broadcast multiply)
  outT (f, q) = accum_k V_h^T-slices @ attn_h   (2 heads packed per matmul
    via column groups)
  y (q, e) = outT^T @ w_out + ones^T b_out, DMA'd out per q-block

All PSUM users (score pairs, attnV waves, projections, out-proj) share one
2-slot ring of 4-bank tiles.  The emission order software-pipelines
everything: K/Q projection tiles are interleaved with q-block-0 score
pairs so the scalar engine starts exp'ing early; attnV waves / out-proj /
V-proj chunks are emitted as PE "filler" between later score pairs so the
PE never idles long enough to lose the HAM 2.4GHz clock.
"""

import copy as _copy
from collections import deque

import numpy as np
import ml_dtypes

import concourse.bass as bass
import concourse.mybir as mybir
import concourse.tile as tile
from concourse import bacc
from concourse.bass_utils import run_bass_kernel_spmd

F32 = mybir.dt.float32
BF16 = mybir.dt.bfloat16
Exp = mybir.ActivationFunctionType.Exp

S = 1024      # sequence length per core (batch element)
C = 1024      # model dim
H = 16        # heads
HD = 64       # head dim
SCALE = 1.0 / (C ** 0.5)
QB = 128      # q block size
NQB = S // QB          # 8 q blocks
NKT = S // 128         # 8 k tiles
NCT = C // 128         # 8 contraction tiles
NHP = H // 2           # 8 head pairs


def build():
    nc = bacc.Bacc(None, target_bir_lowering=False)
    xT_ext = nc.declare_dram_parameter("xT", [C, S], BF16, isOutput=False)
    wq_ext = nc.declare_dram_parameter("w_q", [C, C], BF16, isOutput=False)
    wk_ext = nc.declare_dram_parameter("w_k", [C, C], BF16, isOutput=False)
    wv_ext = nc.declare_dram_parameter("w_v", [C, C], BF16, isOutput=False)
    wout_ext = nc.declare_dram_parameter("w_out", [C, C], BF16, isOutput=False)
    b_ext = nc.declare_dram_parameter("b_out", [C], F32, isOutput=False)
    out_ext = nc.declare_dram_parameter("out", [S, C], F32, isOutput=True)

    with tile.TileContext(nc) as tc:
        with (
            tc.tile_pool(name="const_p", bufs=1) as const_p,
            tc.tile_pool(name="persist", bufs=1) as persist,
            tc.tile_pool(name="e_pool", bufs=2) as e_pool,
            tc.tile_pool(name="tmp_p", bufs=1) as tmp_p,
            tc.tile_pool(name="o_pool", bufs=2) as o_pool,
            tc.tile_pool(name="ps_sc", bufs=2, space="PSUM") as ps_sc,
            tc.tile_pool(name="ps_big", bufs=2, space="PSUM") as ps_big,
        ):
            # ---- constants + ACT exp-table warm ----
            ones1 = const_p.tile([1, 128], BF16)
            nc.vector.memset(ones1, 1.0)
            dum = const_p.tile([1, 128], BF16)
            nc.scalar.activation(dum, ones1, Exp)  # pull ACT_TABLE_LOAD to t=0
            b_f = const_p.tile([1, C], F32)
            nc.sync.dma_start(b_f, b_ext[None, :])
            b_sb = const_p.tile([1, C], BF16)
            nc.vector.tensor_copy(b_sb, b_f)

            # ---- persistent activations ----
            xT = persist.tile([128, NCT, S], BF16)      # 16 KB/part
            QKT = persist.tile([128, H, S], BF16)       # 32 KB/part
            wv_sb = persist.tile([128, NCT, C], BF16)   # 16 KB/part



            # ---------------- helpers ----------------
            def vcopy(dst, src):
                nc.vector.tensor_copy(dst, src)

            def scopy(dst, src):
                nc.scalar.copy(dst, src)

            def proj_tile(dst, w_sb, ft, eng):
                """QKT[:, dst, :] = (x @ w[:, ft-tile])^T, one 128-row tile."""
                pss = ps_big.tile([128, 2, 512], F32, tag="big",
                                  name=f"pj{dst}")
                for ct in range(NCT):
                    lhsT = w_sb[:, ct, ft * 128:(ft + 1) * 128]
                    for sb in range(2):
                        nc.tensor.matmul(
                            pss[:, sb, :], lhsT,
                            xT[:, ct, sb * 512:(sb + 1) * 512],
                            start=(ct == 0), stop=(ct == NCT - 1))
                for sb in range(2):
                    eng(QKT[:, dst, sb * 512:(sb + 1) * 512], pss[:, sb, :])

            def score_pair(qb, hp, E_t, u_t):
                """scores + exp for heads (2hp, 2hp+1) of q-block qb, plus
                incremental denominator accumulation on DVE every 2 pairs."""
                pss_e = ps_sc.tile([128, S], F32, tag="sc",
                                   name=f"sc{qb}_{hp}e")
                pss_o = ps_sc.tile([128, S], F32, tag="sc",
                                   name=f"sc{qb}_{hp}o")
                for kt in range(NKT):
                    for po, pss in ((0, pss_e), (64, pss_o)):
                        nc.tensor.matmul(
                            pss[:, kt * 128:(kt + 1) * 128],
                            QKT[po:po + 64, 8 + hp, kt * 128:(kt + 1) * 128],
                            QKT[po:po + 64, hp, qb * QB:(qb + 1) * QB],
                            start=True, stop=True)
                nc.scalar.activation(E_t[:, 2 * hp, :], pss_e, Exp,
                                     scale=SCALE)
                nc.scalar.activation(E_t[:, 2 * hp + 1, :], pss_o, Exp,
                                     scale=SCALE)
                # incremental denominator: after pairs 3/5/7 fold 4 more
                # head tiles into the accumulator so only a short chain
                # remains after the last exp
                if hp == 3:
                    nc.vector.tensor_add(u_t, E_t[:, 0:4, :], E_t[:, 4:8, :])
                elif hp == 5:
                    nc.vector.tensor_add(u_t, u_t, E_t[:, 8:12, :])
                elif hp == 7:
                    nc.vector.tensor_add(u_t, u_t, E_t[:, 12:16, :])

            def denom_norm(qb, E_t, u_t):
                """finish D = sum_h E, attn = E * (1/D) in place (normalize
                per head pair so attnV waves can start early)."""
                nc.vector.tensor_add(u_t[:, 0:2, :], u_t[:, 0:2, :],
                                     u_t[:, 2:4, :])
                denf = tmp_p.tile([128, S], F32, tag="denf", name=f"denf{qb}")
                nc.vector.tensor_add(denf, u_t[:, 0, :], u_t[:, 1, :])
                recf = tmp_p.tile([128, S], F32, tag="recf", name=f"recf{qb}")
                nc.vector.reciprocal_approx_fast(out=recf, in_=denf)
                rec = tmp_p.tile([128, S], BF16, tag="rec", name=f"rec{qb}")
                nc.vector.tensor_copy(rec, recf)
                rb = rec[:, :].unsqueeze(1).broadcast_to((128, 2, S))
                for hp in range(NHP):
                    nc.vector.tensor_mul(E_t[:, 2 * hp:2 * hp + 2, :],
                                         E_t[:, 2 * hp:2 * hp + 2, :], rb)

            def attnv_wave(qb, w, E_t, outT_t):
                """attn @ v for heads (2w, 2w+1), packed via column groups."""
                aw = ps_big.tile([128, 2, 512], F32, tag="big",
                                 name=f"aw{qb}_{w}")
                for kt in range(NKT):
                    for i in (0, 1):
                        h = 2 * w + i
                        po = 64 * i
                        nc.tensor.matmul(
                            aw[po:po + 64, i, 0:QB],
                            Vb[:, kt, h * HD:(h + 1) * HD],
                            E_t[:, h, kt * 128:(kt + 1) * 128],
                            start=(kt == 0), stop=(kt == NKT - 1),
                            tile_position=(0, po))
                nc.scalar.copy(outT_t[0:64, w, :], aw[0:64, 0, 0:QB])
                nc.scalar.copy(outT_t[64:128, w, :], aw[64:128, 1, 0:QB])

            def out_proj_half(qb, outT_t, ec):
                psy = ps_big.tile([128, 2, 512], F32, tag="big",
                                  name=f"psy{qb}_{ec}")
                for ft in range(NCT):
                    nc.tensor.matmul(
                        psy[:, 0, :], outT_t[:, ft, :],
                        wout_sb[:, ft, ec * 512:(ec + 1) * 512],
                        start=(ft == 0), stop=False)
                nc.tensor.matmul(
                    psy[:, 0, :], ones1, b_sb[:, ec * 512:(ec + 1) * 512],
                    start=False, stop=True)
                y_t = o_pool.tile([128, 512], F32, tag="y", name=f"y{qb}_{ec}")
                nc.scalar.copy(y_t, psy[:, 0, :])
                nc.sync.dma_start(
                    out_ext[qb * QB:(qb + 1) * QB, ec * 512:(ec + 1) * 512],
                    y_t)

            def v_chunk(st, eng0, eng1):
                """Vb[:, st, :] = x-rows-st @ w_v  (one 128-row slab)."""
                pss = ps_big.tile([128, 2, 512], F32, tag="big",
                                  name=f"vp{st}")
                for ct in range(NCT):
                    lhsT = xT[:, ct, st * 128:(st + 1) * 128]
                    for fb in range(2):
                        nc.tensor.matmul(
                            pss[:, fb, :], lhsT,
                            wv_sb[:, ct, fb * 512:(fb + 1) * 512],
                            start=(ct == 0), stop=(ct == NCT - 1))
                eng0(Vb[:, st, 0:512], pss[:, 0, :])
                eng1(Vb[:, st, 512:1024], pss[:, 1, :])

            def new_E(qb):
                E_t = e_pool.tile([128, H, S], BF16, tag="E", name=f"E{qb}")
                u_t = tmp_p.tile([128, 4, S], BF16, tag="u", name=f"u{qb}")
                return E_t, u_t

            # ============ lead-in: K/Q projection ∥ q-block 0 ============
            E0 = new_E(0)
            with tc.tile_pool(name="wkq_p", bufs=1) as wkq_p:
                wk_sb = wkq_p.tile([128, NCT, C], BF16)
                wq_sb = wkq_p.tile([128, NCT, C], BF16)
                # interleave wk / xT tile loads so the first K-proj matmul
                # can start as early as possible
                for ct in range(NCT):
                    nc.sync.dma_start(
                        wk_sb[:, ct, :], wk_ext[ct * 128:(ct + 1) * 128, :])
                    nc.scalar.dma_start(
                        xT[:, ct, :], xT_ext[ct * 128:(ct + 1) * 128, :])
                for ct in range(NCT):
                    nc.sync.dma_start(
                        wq_sb[:, ct, :], wq_ext[ct * 128:(ct + 1) * 128, :])
                for ct in range(NCT):
                    nc.sync.dma_start(
                        wv_sb[:, ct, :], wv_ext[ct * 128:(ct + 1) * 128, :])
                with nc.named_scope("kq_proj"):
                    for ft in range(NHP):
                        proj_tile(8 + ft, wk_sb, ft, vcopy)
                        proj_tile(ft, wq_sb, ft, scopy)
                        score_pair(0, ft, *E0)

            # ============ main: q-blocks 1..7 with PE fillers ============
            with tc.tile_pool(name="rest_p", bufs=1) as rest_p:
                Vb = rest_p.tile([128, NKT, C], BF16)
                wout_sb = rest_p.tile([128, NCT, C], BF16)
                for ft in range(NCT):
                    nc.sync.dma_start(
                        wout_sb[:, ft, :], wout_ext[ft * 128:(ft + 1) * 128, :])

                denom_norm(0, *E0)

                # three filler queues with different readiness lags:
                #  vq: V-proj chunks (ready immediately) - first half of qb1
                #  wq: attnV waves, lag 1 - second half of each qb (their
                #      normalize chain finishes early in the qb)
                #  pq: out-proj halves, lag 2 - first pairs of each qb
                #      (always ready, keeps the PE dense at qb boundaries)
                vq = deque()
                wq = deque()
                pq = deque()
                for st in range(NKT):
                    vq.append(
                        lambda st=st: v_chunk(
                            st, vcopy if st % 2 else scopy,
                            scopy if st % 2 else vcopy))
                outT0 = o_pool.tile([128, NCT, QB], BF16, tag="outT",
                                    name="outT0")
                for w in range(NHP):
                    wq.append(
                        lambda w=w: attnv_wave(0, w, E0[0], outT0))
                for ec in range(2):
                    pq.append(
                        lambda ec=ec: out_proj_half(0, outT0, ec))

                for qb in range(1, NQB):
                    Eq = new_E(qb)
                    with nc.named_scope(f"qb{qb}"):
                        used_v = False
                        for hp in range(NHP):
                            score_pair(qb, hp, *Eq)
                            if vq and hp < 4:
                                vq.popleft()()
                                vq.popleft()()
                                used_v = True
                                continue
                            if hp < 2:
                                if pq:
                                    pq.popleft()()
                            elif hp < 4:
                                if not used_v and wq:
                                    wq.popleft()()
                            elif hp < 7 or used_v:
                                for _ in range(2):
                                    if wq:
                                        wq.popleft()()
                        denom_norm(qb, *Eq)
                    outT_t = o_pool.tile([128, NCT, QB], BF16, tag="outT",
                                         name=f"outT{qb}")
                    for w in range(NHP):
                        wq.append(
                            lambda w=w, Eq=Eq, o=outT_t, q=qb:
                            attnv_wave(q, w, Eq[0], o))
                    for ec in range(2):
                        pq.append(
                            lambda q=qb, o=outT_t, ec=ec:
                            out_proj_half(q, o, ec))

                with nc.named_scope("tail"):
                    while wq:
                        wq.popleft()()
                        if len(pq) > 2:  # qb6's halves between early waves;
                            pq.popleft()()  # qb7's must wait for its outT
                    while pq:
                        pq.popleft()()

    nc.compile()
    return nc


_NC = None


def _get_nc():
    global _NC
    if _NC is None:
        _NC = build()
    return _NC


def make_in_maps(x, w_qkv, w_out, b_out):
    bf = ml_dtypes.bfloat16
    x = np.asarray(x, dtype=np.float32)
    w_qkv = np.asarray(w_qkv, dtype=np.float32)
    wq = np.ascontiguousarray(w_qkv[:, 0:C]).astype(bf)
    wk = np.ascontiguousarray(w_qkv[:, C:2 * C]).astype(bf)
    wv = np.ascontiguousarray(w_qkv[:, 2 * C:3 * C]).astype(bf)
    wo = np.ascontiguousarray(np.asarray(w_out, dtype=np.float32)).astype(bf)
    b = np.ascontiguousarray(np.asarray(b_out, dtype=np.float32))
    return [
        {"xT": np.ascontiguousarray(x[i].T).astype(bf), "w_q": wq, "w_k": wk,
         "w_v": wv, "w_out": wo, "b_out": b}
        for i in range(8)
    ]


def kernel(x, w_qkv, w_out, b_out):
    nc = _get_nc()
    in_maps = make_in_maps(x, w_qkv, w_out, b_out)
    res = run_bass_kernel_spmd(nc, in_maps, core_ids=list(range(8)))
    out = np.stack([np.asarray(res.results[i]["out"]) for i in range(8)])
    return out.astype(np.float32)


# revision 56
# speedup vs baseline: 1.1801x; 1.1801x over previous
"""Distributed Trainium2 kernel for nn_Attention_21990232555717.

Reference (per batch element a, seq s=1024, model dim c=1024, 16 heads):
    qkv = x @ w_qkv                       # (s, 3072)
    scores = q @ k.T * (1/sqrt(1024))     # (h, s, s)
    attn = softmax(scores, axis=HEADS)    # normalize across the 16 heads
    out = attn @ v -> (s, 1024) @ w_out + b_out

Sharding: pure data parallel - batch (8) across 8 cores, weights replicated.

Per-core dataflow, all bf16 on the matmul paths (inputs converted to bf16
and x pre-transposed on the host, f32 accumulation in PSUM):
  xT   (c, s)  loaded directly (host supplies x^T)
  QKT  (f, s)  = w^T @ x^T    Q tiles 0..7, K tiles 8..15 (2 heads/tile)
  Vb   (s, f)  = x @ w_v
  per q-block of 128 (8 blocks), per head pair: scoresT (k,q) for both
    heads land in one 4-bank PSUM tile -> ONE exp over FD=2048 into
    E[128, 16, 1024] (heads interleaved)
    D = sum_h E via one DVE tree; attn = E * recip(D) (broadcast multiply)
  outT (f, q) = accum_k V_h^T-slices @ attn_h   (2 heads packed per matmul
    via column groups)
  y (q, e) = outT^T @ w_out + ones^T b_out, DMA'd out per q-block

All PSUM users (score pairs, attnV waves, projections, out-proj) share one
2-slot ring of 4-bank tiles.  The emission order software-pipelines
everything: K/Q projection tiles are interleaved with q-block-0 score
pairs so the scalar engine starts exp'ing early; attnV waves / out-proj /
V-proj chunks are emitted as PE "filler" between later score pairs so the
PE never idles long enough to lose the HAM 2.4GHz clock.
"""

import copy as _copy
from collections import deque

import numpy as np
import ml_dtypes

import concourse.bass as bass
import concourse.mybir as mybir
import concourse.tile as tile
from concourse import bacc
from concourse.bass_utils import run_bass_kernel_spmd

F32 = mybir.dt.float32
BF16 = mybir.dt.bfloat16
Exp = mybir.ActivationFunctionType.Exp

S = 1024      # sequence length per core (batch element)
C = 1024      # model dim
H = 16        # heads
HD = 64       # head dim
SCALE = 1.0 / (C ** 0.5)
QB = 128      # q block size
NQB = S // QB          # 8 q blocks
NKT = S // 128         # 8 k tiles
NCT = C // 128         # 8 contraction tiles
NHP = H // 2           # 8 head pairs


def build():
    nc = bacc.Bacc(None, target_bir_lowering=False)
    xT_ext = nc.declare_dram_parameter("xT", [C, S], BF16, isOutput=False)
    wq_ext = nc.declare_dram_parameter("w_q", [C, C], BF16, isOutput=False)
    wk_ext = nc.declare_dram_parameter("w_k", [C, C], BF16, isOutput=False)
    wv_ext = nc.declare_dram_parameter("w_v", [C, C], BF16, isOutput=False)
    wout_ext = nc.declare_dram_parameter("w_out", [C, C], BF16, isOutput=False)
    b_ext = nc.declare_dram_parameter("b_out", [C], F32, isOutput=False)
    out_ext = nc.declare_dram_parameter("out", [S, C], F32, isOutput=True)

    with tile.TileContext(nc) as tc:
        with (
            tc.tile_pool(name="const_p", bufs=1) as const_p,
            tc.tile_pool(name="persist", bufs=1) as persist,
            tc.tile_pool(name="e_pool", bufs=2) as e_pool,
            tc.tile_pool(name="tmp_p", bufs=1) as tmp_p,
            tc.tile_pool(name="o_pool", bufs=2) as o_pool,
            tc.tile_pool(name="ps_sc", bufs=2, space="PSUM") as ps_sc,
            tc.tile_pool(name="ps_big", bufs=2, space="PSUM") as ps_big,
        ):
            # ---- constants + ACT exp-table warm ----
            ones1 = const_p.tile([1, 128], BF16)
            nc.vector.memset(ones1, 1.0)
            dum = const_p.tile([1, 128], BF16)
            nc.scalar.activation(dum, ones1, Exp)  # pull ACT_TABLE_LOAD to t=0
            b_f = const_p.tile([1, C], F32)
            nc.sync.dma_start(b_f, b_ext[None, :])
            b_sb = const_p.tile([1, C], BF16)
            nc.vector.tensor_copy(b_sb, b_f)
            wsrc = const_p.tile([1, 512], BF16)
            nc.vector.memset(wsrc, 1.0)

            # ---- persistent activations ----
            xT = persist.tile([128, NCT, S], BF16)      # 16 KB/part
            QKT = persist.tile([128, H, S], BF16)       # 32 KB/part
            wv_sb = persist.tile([128, NCT, C], BF16)   # 16 KB/part



            # ---------------- helpers ----------------
            def vcopy(dst, src):
                nc.vector.tensor_copy(dst, src)

            def scopy(dst, src):
                nc.scalar.copy(dst, src)

            def proj_tile(dst, w_sb, ft, eng, keepalive=False):
                """QKT[:, dst, :] = (x @ w[:, ft-tile])^T, one 128-row tile."""
                pss = ps_big.tile([128, 2, 512], F32, tag="big",
                                  name=f"pj{dst}")
                for ct in range(NCT):
                    lhsT = w_sb[:, ct, ft * 128:(ft + 1) * 128]
                    for sb in range(2):
                        nc.tensor.matmul(
                            pss[:, sb, :], lhsT,
                            xT[:, ct, sb * 512:(sb + 1) * 512],
                            start=(ct == 0), stop=(ct == NCT - 1))
                for sb in range(2):
                    eng(QKT[:, dst, sb * 512:(sb + 1) * 512], pss[:, sb, :])

            def score_pair(qb, hp, E_t, u_t):
                """scores + exp for heads (2hp, 2hp+1) of q-block qb, plus
                incremental denominator accumulation on DVE every 2 pairs."""
                pss_e = ps_sc.tile([128, S], F32, tag="sc",
                                   name=f"sc{qb}_{hp}e")
                pss_o = ps_sc.tile([128, S], F32, tag="sc",
                                   name=f"sc{qb}_{hp}o")
                for kt in range(NKT):
                    for po, pss in ((0, pss_e), (64, pss_o)):
                        nc.tensor.matmul(
                            pss[:, kt * 128:(kt + 1) * 128],
                            QKT[po:po + 64, 8 + hp, kt * 128:(kt + 1) * 128],
                            QKT[po:po + 64, hp, qb * QB:(qb + 1) * QB],
                            start=True, stop=True)
                nc.scalar.activation(E_t[:, 2 * hp, :], pss_e, Exp,
                                     scale=SCALE)
                nc.scalar.activation(E_t[:, 2 * hp + 1, :], pss_o, Exp,
                                     scale=SCALE)
                # incremental denominator: after pairs 3/5/7 fold 4 more
                # head tiles into the accumulator so only a short chain
                # remains after the last exp
                if hp == 3:
                    nc.vector.tensor_add(u_t, E_t[:, 0:4, :], E_t[:, 4:8, :])
                elif hp == 5:
                    nc.vector.tensor_add(u_t, u_t, E_t[:, 8:12, :])
                elif hp == 7:
                    nc.vector.tensor_add(u_t, u_t, E_t[:, 12:16, :])

            def denom_norm(qb, E_t, u_t):
                """finish D = sum_h E, attn = E * (1/D) in place (normalize
                per head pair so attnV waves can start early)."""
                nc.vector.tensor_add(u_t[:, 0:2, :], u_t[:, 0:2, :],
                                     u_t[:, 2:4, :])
                denf = tmp_p.tile([128, S], F32, tag="denf", name=f"denf{qb}")
                nc.vector.tensor_add(denf, u_t[:, 0, :], u_t[:, 1, :])
                recf = tmp_p.tile([128, S], F32, tag="recf", name=f"recf{qb}")
                nc.vector.reciprocal_approx_fast(out=recf, in_=denf)
                rec = tmp_p.tile([128, S], BF16, tag="rec", name=f"rec{qb}")
                nc.vector.tensor_copy(rec, recf)
                rb = rec[:, :].unsqueeze(1).broadcast_to((128, 2, S))
                for hp in range(NHP):
                    nc.vector.tensor_mul(E_t[:, 2 * hp:2 * hp + 2, :],
                                         E_t[:, 2 * hp:2 * hp + 2, :], rb)

            def attnv_wave(qb, w, E_t, outT_t):
                """attn @ v for heads (2w, 2w+1), packed via column groups."""
                aw = ps_big.tile([128, 2, 512], F32, tag="big",
                                 name=f"aw{qb}_{w}")
                for kt in range(NKT):
                    for i in (0, 1):
                        h = 2 * w + i
                        po = 64 * i
                        nc.tensor.matmul(
                            aw[po:po + 64, i, 0:QB],
                            Vb[:, kt, h * HD:(h + 1) * HD],
                            E_t[:, h, kt * 128:(kt + 1) * 128],
                            start=(kt == 0), stop=(kt == NKT - 1),
                            tile_position=(0, po))
                nc.scalar.copy(outT_t[0:64, w, :], aw[0:64, 0, 0:QB])
                nc.scalar.copy(outT_t[64:128, w, :], aw[64:128, 1, 0:QB])

            def out_proj_half(qb, outT_t, ec):
                psy = ps_big.tile([128, 2, 512], F32, tag="big",
                                  name=f"psy{qb}_{ec}")
                for ft in range(NCT):
                    nc.tensor.matmul(
                        psy[:, 0, :], outT_t[:, ft, :],
                        wout_sb[:, ft, ec * 512:(ec + 1) * 512],
                        start=(ft == 0), stop=False)
                nc.tensor.matmul(
                    psy[:, 0, :], ones1, b_sb[:, ec * 512:(ec + 1) * 512],
                    start=False, stop=True)
                y_t = o_pool.tile([128, 512], F32, tag="y", name=f"y{qb}_{ec}")
                nc.vector.tensor_copy(y_t, psy[:, 0, :])
                nc.sync.dma_start(
                    out_ext[qb * QB:(qb + 1) * QB, ec * 512:(ec + 1) * 512],
                    y_t)

            def v_chunk(st, eng0, eng1):
                """Vb[:, st, :] = x-rows-st @ w_v  (one 128-row slab)."""
                pss = ps_big.tile([128, 2, 512], F32, tag="big",
                                  name=f"vp{st}")
                for ct in range(NCT):
                    lhsT = xT[:, ct, st * 128:(st + 1) * 128]
                    for fb in range(2):
                        nc.tensor.matmul(
                            pss[:, fb, :], lhsT,
                            wv_sb[:, ct, fb * 512:(fb + 1) * 512],
                            start=(ct == 0), stop=(ct == NCT - 1))
                eng0(Vb[:, st, 0:512], pss[:, 0, :])
                eng1(Vb[:, st, 512:1024], pss[:, 1, :])

            def new_E(qb):
                E_t = e_pool.tile([128, H, S], BF16, tag="E", name=f"E{qb}")
                u_t = tmp_p.tile([128, 4, S], BF16, tag="u", name=f"u{qb}")
                return E_t, u_t

            # ============ lead-in: K/Q projection ∥ q-block 0 ============
            E0 = new_E(0)
            with tc.tile_pool(name="wkq_p", bufs=1) as wkq_p:
                wk_sb = wkq_p.tile([128, NCT, C], BF16)
                wq_sb = wkq_p.tile([128, NCT, C], BF16)
                # interleave wk / xT tile loads so the first K-proj matmul
                # can start as early as possible
                for ct in range(NCT):
                    nc.sync.dma_start(
                        wk_sb[:, ct, :], wk_ext[ct * 128:(ct + 1) * 128, :])
                    nc.sync.dma_start(
                        xT[:, ct, :], xT_ext[ct * 128:(ct + 1) * 128, :])
                for ct in range(NCT):
                    nc.sync.dma_start(
                        wq_sb[:, ct, :], wq_ext[ct * 128:(ct + 1) * 128, :])
                for ct in range(NCT):
                    nc.sync.dma_start(
                        wv_sb[:, ct, :], wv_ext[ct * 128:(ct + 1) * 128, :])
                with nc.named_scope("kq_proj"):
                    for ft in range(NHP):
                        proj_tile(8 + ft, wk_sb, ft, vcopy,
                                  keepalive=(ft == 0))
                        proj_tile(ft, wq_sb, ft, scopy)
                        score_pair(0, ft, *E0)

            # ============ main: q-blocks 1..7 with PE fillers ============
            with tc.tile_pool(name="rest_p", bufs=1) as rest_p:
                Vb = rest_p.tile([128, NKT, C], BF16)
                wout_sb = rest_p.tile([128, NCT, C], BF16)
                for ft in range(NCT):
                    nc.sync.dma_start(
                        wout_sb[:, ft, :], wout_ext[ft * 128:(ft + 1) * 128, :])

                denom_norm(0, *E0)

                # three filler queues with different readiness lags:
                #  vq: V-proj chunks (ready immediately) - first half of qb1
                #  wq: attnV waves, lag 1 - second half of each qb (their
                #      normalize chain finishes early in the qb)
                #  pq: out-proj halves, lag 2 - first pairs of each qb
                #      (always ready, keeps the PE dense at qb boundaries)
                vq = deque()
                wq = deque()
                pq = deque()
                for st in range(NKT):
                    vq.append(
                        lambda st=st: v_chunk(
                            st, vcopy if st % 2 else scopy,
                            scopy if st % 2 else vcopy))
                outT0 = o_pool.tile([128, NCT, QB], BF16, tag="outT",
                                    name="outT0")
                for w in range(NHP):
                    wq.append(
                        lambda w=w: attnv_wave(0, w, E0[0], outT0))
                for ec in range(2):
                    pq.append(
                        lambda ec=ec: out_proj_half(0, outT0, ec))

                for qb in range(1, NQB):
                    Eq = new_E(qb)
                    with nc.named_scope(f"qb{qb}"):
                        used_v = False
                        for hp in range(NHP):
                            score_pair(qb, hp, *Eq)
                            if vq and hp < 4:
                                vq.popleft()()
                                vq.popleft()()
                                used_v = True
                                continue
                            if hp < 2:
                                if pq:
                                    pq.popleft()()
                            elif hp < 4:
                                if not used_v and wq:
                                    wq.popleft()()
                            elif hp < 7 or used_v:
                                for _ in range(2):
                                    if wq:
                                        wq.popleft()()
                        denom_norm(qb, *Eq)
                    outT_t = o_pool.tile([128, NCT, QB], BF16, tag="outT",
                                         name=f"outT{qb}")
                    for w in range(NHP):
                        wq.append(
                            lambda w=w, Eq=Eq, o=outT_t, q=qb:
                            attnv_wave(q, w, Eq[0], o))
                    for ec in range(2):
                        pq.append(
                            lambda q=qb, o=outT_t, ec=ec:
                            out_proj_half(q, o, ec))

                with nc.named_scope("tail"):
                    while wq:
                        wq.popleft()()
                        if len(pq) > 2:  # qb6's halves between early waves;
                            pq.popleft()()  # qb7's must wait for its outT
                    while pq:
                        pq.popleft()()

    nc.compile()
    return nc


_NC = None


def _get_nc():
    global _NC
    if _NC is None:
        _NC = build()
    return _NC


def make_in_maps(x, w_qkv, w_out, b_out):
    bf = ml_dtypes.bfloat16
    x = np.asarray(x, dtype=np.float32)
    w_qkv = np.asarray(w_qkv, dtype=np.float32)
    wq = np.ascontiguousarray(w_qkv[:, 0:C]).astype(bf)
    wk = np.ascontiguousarray(w_qkv[:, C:2 * C]).astype(bf)
    wv = np.ascontiguousarray(w_qkv[:, 2 * C:3 * C]).astype(bf)
    wo = np.ascontiguousarray(np.asarray(w_out, dtype=np.float32)).astype(bf)
    b = np.ascontiguousarray(np.asarray(b_out, dtype=np.float32))
    return [
        {"xT": np.ascontiguousarray(x[i].T).astype(bf), "w_q": wq, "w_k": wk,
         "w_v": wv, "w_out": wo, "b_out": b}
        for i in range(8)
    ]


def kernel(x, w_qkv, w_out, b_out):
    nc = _get_nc()
    in_maps = make_in_maps(x, w_qkv, w_out, b_out)
    res = run_bass_kernel_spmd(nc, in_maps, core_ids=list(range(8)))
    out = np.stack([np.asarray(res.results[i]["out"]) for i in range(8)])
    return out.astype(np.float32)


# revision 58
# speedup vs baseline: 1.2009x; 1.0176x over previous
"""Distributed Trainium2 kernel for nn_Attention_21990232555717.

Reference (per batch element a, seq s=1024, model dim c=1024, 16 heads):
    qkv = x @ w_qkv                       # (s, 3072)
    scores = q @ k.T * (1/sqrt(1024))     # (h, s, s)
    attn = softmax(scores, axis=HEADS)    # normalize across the 16 heads
    out = attn @ v -> (s, 1024) @ w_out + b_out

Sharding: pure data parallel - batch (8) across 8 cores, weights replicated.

Per-core dataflow, all bf16 on the matmul paths (inputs converted to bf16
and x pre-transposed on the host, f32 accumulation in PSUM):
  xT   (c, s)  loaded directly (host supplies x^T)
  QKT  (f, s)  = w^T @ x^T    Q tiles 0..7, K tiles 8..15 (2 heads/tile)
  Vb   (s, f)  = x @ w_v
  per q-block of 128 (8 blocks), per head pair: scoresT (k,q) for both
    heads land in one 4-bank PSUM tile -> ONE exp over FD=2048 into
    E[128, 16, 1024] (heads interleaved)
    D = sum_h E via one DVE tree; attn = E * recip(D) (broadcast multiply)
  outT (f, q) = accum_k V_h^T-slices @ attn_h   (2 heads packed per matmul
    via column groups)
  y (q, e) = outT^T @ w_out + ones^T b_out, DMA'd out per q-block

All PSUM users (score pairs, attnV waves, projections, out-proj) share one
2-slot ring of 4-bank tiles.  The emission order software-pipelines
everything: K/Q projection tiles are interleaved with q-block-0 score
pairs so the scalar engine starts exp'ing early; attnV waves / out-proj /
V-proj chunks are emitted as PE "filler" between later score pairs so the
PE never idles long enough to lose the HAM 2.4GHz clock.
"""

import copy as _copy
from collections import deque

import numpy as np
import ml_dtypes

import concourse.bass as bass
import concourse.mybir as mybir
import concourse.tile as tile
from concourse import bacc
from concourse.bass_utils import run_bass_kernel_spmd

F32 = mybir.dt.float32
BF16 = mybir.dt.bfloat16
Exp = mybir.ActivationFunctionType.Exp

S = 1024      # sequence length per core (batch element)
C = 1024      # model dim
H = 16        # heads
HD = 64       # head dim
SCALE = 1.0 / (C ** 0.5)
QB = 128      # q block size
NQB = S // QB          # 8 q blocks
NKT = S // 128         # 8 k tiles
NCT = C // 128         # 8 contraction tiles
NHP = H // 2           # 8 head pairs


def build():
    nc = bacc.Bacc(None, target_bir_lowering=False)
    xT_ext = nc.declare_dram_parameter("xT", [C, S], BF16, isOutput=False)
    wq_ext = nc.declare_dram_parameter("w_q", [C, C], BF16, isOutput=False)
    wk_ext = nc.declare_dram_parameter("w_k", [C, C], BF16, isOutput=False)
    wv_ext = nc.declare_dram_parameter("w_v", [C, C], BF16, isOutput=False)
    wout_ext = nc.declare_dram_parameter("w_out", [C, C], BF16, isOutput=False)
    b_ext = nc.declare_dram_parameter("b_out", [C], F32, isOutput=False)
    out_ext = nc.declare_dram_parameter("out", [S, C], F32, isOutput=True)

    with tile.TileContext(nc) as tc:
        with (
            tc.tile_pool(name="const_p", bufs=1) as const_p,
            tc.tile_pool(name="persist", bufs=1) as persist,
            tc.tile_pool(name="e_pool", bufs=2) as e_pool,
            tc.tile_pool(name="tmp_p", bufs=1) as tmp_p,
            tc.tile_pool(name="o_pool", bufs=2) as o_pool,
            tc.tile_pool(name="ps_sc", bufs=2, space="PSUM") as ps_sc,
            tc.tile_pool(name="ps_big", bufs=2, space="PSUM") as ps_big,
        ):
            # ---- constants + ACT exp-table warm ----
            ones1 = const_p.tile([1, 128], BF16)
            nc.vector.memset(ones1, 1.0)
            dum = const_p.tile([1, 128], BF16)
            nc.scalar.activation(dum, ones1, Exp)  # pull ACT_TABLE_LOAD to t=0
            b_f = const_p.tile([1, C], F32)
            nc.sync.dma_start(b_f, b_ext[None, :])
            b_sb = const_p.tile([1, C], BF16)
            nc.vector.tensor_copy(b_sb, b_f)
            wsrc = const_p.tile([1, 512], BF16)
            nc.vector.memset(wsrc, 1.0)

            # ---- persistent activations ----
            xT = persist.tile([128, NCT, S], BF16)      # 16 KB/part
            QKT = persist.tile([128, H, S], BF16)       # 32 KB/part
            wv_sb = persist.tile([128, NCT, C], BF16)   # 16 KB/part



            # ---------------- helpers ----------------
            def vcopy(dst, src):
                nc.vector.tensor_copy(dst, src)

            def scopy(dst, src):
                nc.scalar.copy(dst, src)

            def proj_tile(dst, w_sb, ft, eng, keepalive=False):
                """QKT[:, dst, :] = (x @ w[:, ft-tile])^T, one 128-row tile."""
                pss = ps_big.tile([128, 2, 512], F32, tag="big",
                                  name=f"pj{dst}")
                for ct in range(NCT):
                    lhsT = w_sb[:, ct, ft * 128:(ft + 1) * 128]
                    for sb in range(2):
                        nc.tensor.matmul(
                            pss[:, sb, :], lhsT,
                            xT[:, ct, sb * 512:(sb + 1) * 512],
                            start=(ct == 0), stop=(ct == NCT - 1))
                for sb in range(2):
                    eng(QKT[:, dst, sb * 512:(sb + 1) * 512], pss[:, sb, :])

            def score_pair(qb, hp, E_t, u_t):
                """scores + exp for heads (2hp, 2hp+1) of q-block qb, plus
                incremental denominator accumulation on DVE every 2 pairs."""
                pss_e = ps_sc.tile([128, S], F32, tag="sc",
                                   name=f"sc{qb}_{hp}e")
                pss_o = ps_sc.tile([128, S], F32, tag="sc",
                                   name=f"sc{qb}_{hp}o")
                for kt in range(NKT):
                    for po, pss in ((0, pss_e), (64, pss_o)):
                        nc.tensor.matmul(
                            pss[:, kt * 128:(kt + 1) * 128],
                            QKT[po:po + 64, 8 + hp, kt * 128:(kt + 1) * 128],
                            QKT[po:po + 64, hp, qb * QB:(qb + 1) * QB],
                            start=True, stop=True)
                nc.scalar.activation(E_t[:, 2 * hp, :], pss_e, Exp,
                                     scale=SCALE)
                nc.scalar.activation(E_t[:, 2 * hp + 1, :], pss_o, Exp,
                                     scale=SCALE)
                # incremental denominator: after pairs 3/5/7 fold 4 more
                # head tiles into the accumulator so only a short chain
                # remains after the last exp
                if hp == 3:
                    nc.vector.tensor_add(u_t, E_t[:, 0:4, :], E_t[:, 4:8, :])
                elif hp == 5:
                    nc.vector.tensor_add(u_t, u_t, E_t[:, 8:12, :])
                elif hp == 7:
                    nc.vector.tensor_add(u_t, u_t, E_t[:, 12:16, :])

            def denom_norm(qb, E_t, u_t):
                """finish D = sum_h E, attn = E * (1/D) in place (normalize
                per head pair so attnV waves can start early)."""
                nc.vector.tensor_add(u_t[:, 0:2, :], u_t[:, 0:2, :],
                                     u_t[:, 2:4, :])
                denf = tmp_p.tile([128, S], F32, tag="denf", name=f"denf{qb}")
                nc.vector.tensor_add(denf, u_t[:, 0, :], u_t[:, 1, :])
                recf = tmp_p.tile([128, S], F32, tag="recf", name=f"recf{qb}")
                nc.vector.reciprocal_approx_fast(out=recf, in_=denf)
                rec = tmp_p.tile([128, S], BF16, tag="rec", name=f"rec{qb}")
                nc.vector.tensor_copy(rec, recf)
                rb = rec[:, :].unsqueeze(1).broadcast_to((128, 2, S))
                for hp in range(NHP):
                    nc.vector.tensor_mul(E_t[:, 2 * hp:2 * hp + 2, :],
                                         E_t[:, 2 * hp:2 * hp + 2, :], rb)

            def attnv_wave(qb, w, E_t, outT_t):
                """attn @ v for heads (2w, 2w+1), packed via column groups."""
                aw = ps_big.tile([128, 2, 512], F32, tag="big",
                                 name=f"aw{qb}_{w}")
                for kt in range(NKT):
                    for i in (0, 1):
                        h = 2 * w + i
                        po = 64 * i
                        nc.tensor.matmul(
                            aw[po:po + 64, i, 0:QB],
                            Vb[:, kt, h * HD:(h + 1) * HD],
                            E_t[:, h, kt * 128:(kt + 1) * 128],
                            start=(kt == 0), stop=(kt == NKT - 1),
                            tile_position=(0, po))
                nc.scalar.copy(outT_t[0:64, w, :], aw[0:64, 0, 0:QB])
                nc.scalar.copy(outT_t[64:128, w, :], aw[64:128, 1, 0:QB])

            def out_proj_half(qb, outT_t, ec):
                psy = ps_big.tile([128, 2, 512], F32, tag="big",
                                  name=f"psy{qb}_{ec}")
                for ft in range(NCT):
                    nc.tensor.matmul(
                        psy[:, 0, :], outT_t[:, ft, :],
                        wout_sb[:, ft, ec * 512:(ec + 1) * 512],
                        start=(ft == 0), stop=False)
                nc.tensor.matmul(
                    psy[:, 0, :], ones1, b_sb[:, ec * 512:(ec + 1) * 512],
                    start=False, stop=True)
                y_t = o_pool.tile([128, 512], F32, tag="y", name=f"y{qb}_{ec}")
                nc.scalar.copy(y_t, psy[:, 0, :])
                nc.sync.dma_start(
                    out_ext[qb * QB:(qb + 1) * QB, ec * 512:(ec + 1) * 512],
                    y_t)

            def v_chunk(st, eng0, eng1):
                """Vb[:, st, :] = x-rows-st @ w_v  (one 128-row slab)."""
                pss = ps_big.tile([128, 2, 512], F32, tag="big",
                                  name=f"vp{st}")
                for ct in range(NCT):
                    lhsT = xT[:, ct, st * 128:(st + 1) * 128]
                    for fb in range(2):
                        nc.tensor.matmul(
                            pss[:, fb, :], lhsT,
                            wv_sb[:, ct, fb * 512:(fb + 1) * 512],
                            start=(ct == 0), stop=(ct == NCT - 1))
                eng0(Vb[:, st, 0:512], pss[:, 0, :])
                eng1(Vb[:, st, 512:1024], pss[:, 1, :])

            def new_E(qb):
                E_t = e_pool.tile([128, H, S], BF16, tag="E", name=f"E{qb}")
                u_t = tmp_p.tile([128, 4, S], BF16, tag="u", name=f"u{qb}")
                return E_t, u_t

            # ============ lead-in: K/Q projection ∥ q-block 0 ============
            E0 = new_E(0)
            with tc.tile_pool(name="wkq_p", bufs=1) as wkq_p:
                wk_sb = wkq_p.tile([128, NCT, C], BF16)
                wq_sb = wkq_p.tile([128, NCT, C], BF16)
                # interleave wk / xT tile loads so the first K-proj matmul
                # can start as early as possible
                for ct in range(NCT):
                    nc.sync.dma_start(
                        wk_sb[:, ct, :], wk_ext[ct * 128:(ct + 1) * 128, :])
                    nc.sync.dma_start(
                        xT[:, ct, :], xT_ext[ct * 128:(ct + 1) * 128, :])
                for ct in range(NCT):
                    nc.sync.dma_start(
                        wq_sb[:, ct, :], wq_ext[ct * 128:(ct + 1) * 128, :])
                for ct in range(NCT):
                    nc.sync.dma_start(
                        wv_sb[:, ct, :], wv_ext[ct * 128:(ct + 1) * 128, :])
                with nc.named_scope("kq_proj"):
                    # first K and Q tiles: interleave their ct-chains so the
                    # DMA-paced arrival window advances both at once
                    pk = ps_big.tile([128, 2, 512], F32, tag="big", name="pj8")
                    pq0 = ps_big.tile([128, 2, 512], F32, tag="big",
                                      name="pj0")
                    for ct in range(NCT):
                        for w_sb, pss in ((wk_sb, pk), (wq_sb, pq0)):
                            lhsT = w_sb[:, ct, 0:128]
                            for sb in range(2):
                                nc.tensor.matmul(
                                    pss[:, sb, :], lhsT,
                                    xT[:, ct, sb * 512:(sb + 1) * 512],
                                    start=(ct == 0), stop=(ct == NCT - 1))
                    for sb in range(2):
                        vcopy(QKT[:, 8, sb * 512:(sb + 1) * 512], pk[:, sb, :])
                        scopy(QKT[:, 0, sb * 512:(sb + 1) * 512], pq0[:, sb, :])
                    score_pair(0, 0, *E0)
                    for ft in range(1, NHP):
                        proj_tile(8 + ft, wk_sb, ft, vcopy)
                        proj_tile(ft, wq_sb, ft, scopy)
                        score_pair(0, ft, *E0)

            # ============ main: q-blocks 1..7 with PE fillers ============
            with tc.tile_pool(name="rest_p", bufs=1) as rest_p:
                Vb = rest_p.tile([128, NKT, C], BF16)
                wout_sb = rest_p.tile([128, NCT, C], BF16)
                for ft in range(NCT):
                    nc.sync.dma_start(
                        wout_sb[:, ft, :], wout_ext[ft * 128:(ft + 1) * 128, :])

                denom_norm(0, *E0)

                # three filler queues with different readiness lags:
                #  vq: V-proj chunks (ready immediately) - first half of qb1
                #  wq: attnV waves, lag 1 - second half of each qb (their
                #      normalize chain finishes early in the qb)
                #  pq: out-proj halves, lag 2 - first pairs of each qb
                #      (always ready, keeps the PE dense at qb boundaries)
                vq = deque()
                wq = deque()
                pq = deque()
                for st in range(NKT):
                    vq.append(
                        lambda st=st: v_chunk(
                            st, vcopy if st % 2 else scopy,
                            scopy if st % 2 else vcopy))
                outT0 = o_pool.tile([128, NCT, QB], BF16, tag="outT",
                                    name="outT0")
                for w in range(NHP):
                    wq.append(
                        lambda w=w: attnv_wave(0, w, E0[0], outT0))
                for ec in range(2):
                    pq.append(
                        lambda ec=ec: out_proj_half(0, outT0, ec))

                for qb in range(1, NQB):
                    Eq = new_E(qb)
                    with nc.named_scope(f"qb{qb}"):
                        used_v = False
                        for hp in range(NHP):
                            score_pair(qb, hp, *Eq)
                            if vq and hp < 4:
                                vq.popleft()()
                                vq.popleft()()
                                used_v = True
                                continue
                            if hp < 2:
                                if pq:
                                    pq.popleft()()
                            elif hp < 4:
                                if not used_v and wq:
                                    wq.popleft()()
                            elif hp < 7 or used_v:
                                for _ in range(2):
                                    if wq:
                                        wq.popleft()()
                        denom_norm(qb, *Eq)
                    outT_t = o_pool.tile([128, NCT, QB], BF16, tag="outT",
                                         name=f"outT{qb}")
                    for w in range(NHP):
                        wq.append(
                            lambda w=w, Eq=Eq, o=outT_t, q=qb:
                            attnv_wave(q, w, Eq[0], o))
                    for ec in range(2):
                        pq.append(
                            lambda q=qb, o=outT_t, ec=ec:
                            out_proj_half(q, o, ec))

                with nc.named_scope("tail"):
                    while wq:
                        wq.popleft()()
                        if len(pq) > 2:  # qb6's halves between early waves;
                            pq.popleft()()  # qb7's must wait for its outT
                    while pq:
                        pq.popleft()()

    nc.compile()
    return nc


_NC = None


def _get_nc():
    global _NC
    if _NC is None:
        _NC = build()
    return _NC


def make_in_maps(x, w_qkv, w_out, b_out):
    bf = ml_dtypes.bfloat16
    x = np.asarray(x, dtype=np.float32)
    w_qkv = np.asarray(w_qkv, dtype=np.float32)
    wq = np.ascontiguousarray(w_qkv[:, 0:C]).astype(bf)
    wk = np.ascontiguousarray(w_qkv[:, C:2 * C]).astype(bf)
    wv = np.ascontiguousarray(w_qkv[:, 2 * C:3 * C]).astype(bf)
    wo = np.ascontiguousarray(np.asarray(w_out, dtype=np.float32)).astype(bf)
    b = np.ascontiguousarray(np.asarray(b_out, dtype=np.float32))
    return [
        {"xT": np.ascontiguousarray(x[i].T).astype(bf), "w_q": wq, "w_k": wk,
         "w_v": wv, "w_out": wo, "b_out": b}
        for i in range(8)
    ]


def kernel(x, w_qkv, w_out, b_out):
    nc = _get_nc()
    in_maps = make_in_maps(x, w_qkv, w_out, b_out)
    res = run_bass_kernel_spmd(nc, in_maps, core_ids=list(range(8)))
    out = np.stack([np.asarray(res.results[i]["out"]) for i in range(8)])
    return out.astype(np.float32)
